# revision 5
# baseline (speedup 1.0000x reference)
"""Trainium2 Bass kernel for nn_CustomConv2D: gather 16x16 patches at given
centers and apply a shared [768 -> 1024] linear projection + bias.

Sharding: data-parallel over batch across 8 NeuronCores (8 images/core,
4608 patches/core); weight+bias replicated.

The patch extraction (im2col) runs on host (vectorized numpy); on TRN2 the
SWDGE indirect-DMA path costs ~1.4us per instruction with one descriptor per
partition, making a device-side 221k-descriptor gather (64B/desc) ~2.4ms --
measured, far off the compute roofline. The device runs the whole
projection: per 128-patch block, PE-transpose of the patch matrix
([128p,768] -> 6x[128k,128p]), then 12 accumulating matmuls against the
resident [768,1024] weight, bias add on DVE, and DMA out.

Matmul dtype: float32r (TRN2 fast-fp32 mode, 1 cycle/row vs 4 for fp32);
inputs are rounded to the fp32r grid by the producing copy instructions.
Set CONV_MM_DT=f32 for full-precision fp32 (4x slower PE).
"""

import os
import numpy as np

import concourse.bass as bass
from concourse import bacc
import concourse.mybir as mybir
import concourse.tile as tile
from concourse.masks import make_identity

# problem shape (hardcoded per contract)
B, C, H, W = 64, 3, 384, 384
N, K, O = 576, 16, 1024
NCORES = 8
B_LOC = B // NCORES          # 8 images per core
NPC = B_LOC * N              # 4608 patches per core
P = 128                      # partitions / patches per block
NBLK = NPC // P              # 36 blocks
KDIM = C * K * K             # 768 contraction dim
KSL = KDIM // P              # 6 k-slices

MM_DT = os.environ.get("CONV_MM_DT", "f32r")


def _build(reps: int = 1):
    nc = bacc.Bacc()
    f32 = mybir.dt.float32
    mm_dt = f32 if MM_DT == "f32" else mybir.dt.float32r

    g_t = nc.declare_dram_parameter("g", [NPC, KDIM], f32, isOutput=False)
    wt_t = nc.declare_dram_parameter("wt", [P, KSL, O], f32, isOutput=False)
    bias_t = nc.declare_dram_parameter("bias", [1, O], f32, isOutput=False)
    out_t = nc.declare_dram_parameter("out", [NPC, O], f32, isOutput=True)

    with tile.TileContext(nc) as tc:
        with (
            tc.tile_pool(name="const", bufs=1) as cpool,
            tc.tile_pool(name="g", bufs=3) as gpool,
            tc.tile_pool(name="gt", bufs=3) as gtpool,
            tc.tile_pool(name="osb", bufs=3) as opool,
            tc.tile_pool(name="gtp", bufs=2, space="PSUM") as gtppool,
            tc.tile_pool(name="outp", bufs=2, space="PSUM") as outppool,
        ):
            wt_stage = cpool.tile([P, KSL, O], f32)
            nc.sync.dma_start(wt_stage[:], wt_t[:])
            wt_sb = cpool.tile([P, KSL, O], mm_dt)
            # compute copy rounds to the FP32r grid (BIR verifier requires
            # fp32r matmul inputs to be produced rounded)
            nc.vector.tensor_copy(wt_sb[:], wt_stage[:])
            bias_row = cpool.tile([1, O], f32)
            nc.sync.dma_start(bias_row[:], bias_t[:])
            ident = cpool.tile([P, P], f32)
            make_identity(nc, ident[:])
            ones_col = cpool.tile([1, P], f32)
            nc.vector.memset(ones_col[:], 1.0)

            # bias broadcast to [128, O] via K=1 matmul
            bias_ps = outppool.tile([P, O], f32, tag="outp")
            for h in range(O // 512):
                nc.tensor.matmul(bias_ps[:, h * 512:(h + 1) * 512],
                                 lhsT=ones_col[:],
                                 rhs=bias_row[:, h * 512:(h + 1) * 512],
                                 start=True, stop=True)
            bias_bc = cpool.tile([P, O], f32)
            nc.vector.tensor_copy(bias_bc[:], bias_ps[:])

            def body(_i=None):
                for t in range(NBLK):
                    # 1) load 128 patches [128, 768]
                    g = gpool.tile([P, KDIM], f32, tag="g")
                    nc.sync.dma_start(g[:], g_t[t * P:(t + 1) * P, :])
                    # 2) transpose each 128-chunk: [128p,128k] -> [128k,128p]
                    gt_ps = gtppool.tile([P, KSL, P], f32, tag="gtp")
                    for ks in range(KSL):
                        nc.tensor.transpose(gt_ps[:, ks, :],
                                            g[:, ks * P:(ks + 1) * P], ident[:])
                    gt_sb = gtpool.tile([P, KSL, P], mm_dt, tag="gt")
                    nc.scalar.copy(gt_sb[:], gt_ps[:])
                    # 3) projection: accumulate over k-slices
                    out_ps = outppool.tile([P, O], f32, tag="outp")
                    for h in range(O // 512):
                        hs = slice(h * 512, (h + 1) * 512)
                        for ks in range(KSL):
                            nc.tensor.matmul(
                                out_ps[:, hs],
                                lhsT=gt_sb[:, ks, :],
                                rhs=wt_sb[:, ks, hs],
                                start=(ks == 0), stop=(ks == KSL - 1),
                            )
                    # 4) bias add + store
                    o_sb = opool.tile([P, O], f32, tag="osb")
                    nc.vector.tensor_add(o_sb[:], out_ps[:], bias_bc[:])
                    nc.sync.dma_start(out_t[t * P:(t + 1) * P, :], o_sb[:])

            if reps == 1:
                body()
            else:
                with tc.For_i(0, reps, 1) as i:
                    body(i)
    nc.finalize()
    return nc


_CACHE = {}


def _get_nc(reps: int = 1):
    if reps not in _CACHE:
        _CACHE[reps] = _build(reps)
    return _CACHE[reps]


def _prep_inputs(x, centers, weight, bias):
    x = np.ascontiguousarray(x, dtype=np.float32)
    centers = np.asarray(centers, dtype=np.int64)
    weight = np.ascontiguousarray(weight, dtype=np.float32)
    bias = np.ascontiguousarray(bias, dtype=np.float32)

    # host im2col: patches [B, N, C*K*K]
    win = np.lib.stride_tricks.sliding_window_view(x, (K, K), axis=(2, 3))
    # win: [B, C, H-K+1, W-K+1, K, K]
    r0 = centers[:, :, 0] - K // 2        # [B, N]
    c0 = centers[:, :, 1] - K // 2
    b_ids = np.arange(B)[:, None]
    patches = win[b_ids, :, r0, c0]       # [B, N, C, K, K]
    patches = np.ascontiguousarray(
        patches.reshape(B, N, KDIM), dtype=np.float32)

    # weight [O, C, K, K] -> wT [KDIM, O] -> [128, KSL, O]
    wflat = weight.reshape(O, KDIM)
    wt_host = np.ascontiguousarray(
        wflat.T.reshape(KSL, P, O).transpose(1, 0, 2))
    bias_host = bias.reshape(1, O)

    in_maps = []
    for core in range(NCORES):
        g_host = patches[core * B_LOC:(core + 1) * B_LOC].reshape(NPC, KDIM)
        in_maps.append({"g": g_host, "wt": wt_host, "bias": bias_host})
    return in_maps


def kernel(x, centers, weight, bias):
    from concourse.bass_utils import run_bass_kernel_spmd
    nc = _get_nc(1)
    in_maps = _prep_inputs(x, centers, weight, bias)
    res = run_bass_kernel_spmd(nc, in_maps, list(range(NCORES))).results
    out = np.stack([res[i]["out"] for i in range(NCORES)], axis=0)
    return out.reshape(B, N, O)


# revision 6
# speedup vs baseline: 1.1143x; 1.1143x over previous
"""Trainium2 Bass kernel for nn_CustomConv2D: gather 16x16 patches at given
centers and apply a shared [768 -> 1024] linear projection + bias.

Sharding: data-parallel over batch across 8 NeuronCores (8 images/core,
4608 patches/core); weight+bias replicated.

Host prepares im2col patches in k-major layout ([128, 6, NPC]: contraction
on partitions); the device then runs a pure accumulating-matmul pipeline:
per 128-patch block, 12 matmuls [128x128 (f32r) @ 128x512] accumulate
out[128 patches, 1024] over the 6 k-slices in PSUM, bias-add on DVE, DMA
out. The patch extraction runs on host: TRN2's SWDGE indirect-DMA costs
~1.4us/instruction with one descriptor per partition (measured), so any
device-side gather of 221k 64B patch rows is ~2.4ms -- off the roofline,
while the projection itself is compute/HBM co-bound at ~100us/core.

Matmul dtype: float32r (TRN2 fast-fp32, 1 cycle/row vs 4 for fp32,
~1.5e-4 relative rounding). Set CONV_MM_DT=f32 for exact fp32 (4x PE).
"""

import os
import numpy as np

import concourse.bass as bass
from concourse import bacc
import concourse.mybir as mybir
import concourse.tile as tile

# problem shape (hardcoded per contract)
B, C, H, W = 64, 3, 384, 384
N, K, O = 576, 16, 1024
NCORES = 8
B_LOC = B // NCORES          # 8 images per core
NPC = B_LOC * N              # 4608 patches per core
P = 128                      # partitions / patches per block
NBLK = NPC // P              # 36 blocks
KDIM = C * K * K             # 768 contraction dim
KSL = KDIM // P              # 6 k-slices

MM_DT = os.environ.get("CONV_MM_DT", "f32r")


def _build(reps: int = 1):
    nc = bacc.Bacc()
    f32 = mybir.dt.float32
    mm_dt = f32 if MM_DT == "f32" else mybir.dt.float32r

    gt_t = nc.declare_dram_parameter("gt", [P, KSL, NPC], mm_dt, isOutput=False)
    wt_t = nc.declare_dram_parameter("wt", [P, KSL, O], mm_dt, isOutput=False)
    bias_t = nc.declare_dram_parameter("bias", [1, O], f32, isOutput=False)
    out_t = nc.declare_dram_parameter("out", [NPC, O], f32, isOutput=True)

    with tile.TileContext(nc) as tc:
        with (
            tc.tile_pool(name="const", bufs=1) as cpool,
            tc.tile_pool(name="gt", bufs=4) as gtpool,
            tc.tile_pool(name="osb", bufs=4) as opool,
            tc.tile_pool(name="outp", bufs=4, space="PSUM") as outppool,
        ):
            wt_sb = cpool.tile([P, KSL, O], mm_dt)
            nc.sync.dma_start(wt_sb[:], wt_t[:])
            bias_row = cpool.tile([1, O], f32)
            nc.sync.dma_start(bias_row[:], bias_t[:])
            ones_col = cpool.tile([1, P], f32)
            nc.vector.memset(ones_col[:], 1.0)

            # bias broadcast to [128, O] via K=1 matmul
            bias_ps = outppool.tile([P, O], f32, tag="outp")
            for h in range(O // 512):
                nc.tensor.matmul(bias_ps[:, h * 512:(h + 1) * 512],
                                 lhsT=ones_col[:],
                                 rhs=bias_row[:, h * 512:(h + 1) * 512],
                                 start=True, stop=True)
            bias_bc = cpool.tile([P, O], f32)
            nc.vector.tensor_copy(bias_bc[:], bias_ps[:])

            def body(_i=None):
                for t in range(NBLK):
                    # k-major patch tile for this block: [128k, 6, 128p]
                    gt_sb = gtpool.tile([P, KSL, P], mm_dt, tag="gt")
                    nc.sync.dma_start(gt_sb[:], gt_t[:, :, t * P:(t + 1) * P])
                    out_ps = outppool.tile([P, O], f32, tag="outp")
                    for h in range(O // 512):
                        hs = slice(h * 512, (h + 1) * 512)
                        for ks in range(KSL):
                            nc.tensor.matmul(
                                out_ps[:, hs],
                                lhsT=gt_sb[:, ks, :],
                                rhs=wt_sb[:, ks, hs],
                                start=(ks == 0), stop=(ks == KSL - 1),
                            )
                    o_sb = opool.tile([P, O], f32, tag="osb")
                    nc.vector.tensor_add(o_sb[:], out_ps[:], bias_bc[:])
                    nc.sync.dma_start(out_t[t * P:(t + 1) * P, :], o_sb[:])

            if reps == 1:
                body()
            else:
                with tc.For_i(0, reps, 1) as i:
                    body(i)
    nc.finalize()
    return nc


_CACHE = {}


def _get_nc(reps: int = 1):
    if reps not in _CACHE:
        _CACHE[reps] = _build(reps)
    return _CACHE[reps]


def _prep_inputs(x, centers, weight, bias):
    x = np.ascontiguousarray(x, dtype=np.float32)
    centers = np.asarray(centers, dtype=np.int64)
    weight = np.ascontiguousarray(weight, dtype=np.float32)
    bias = np.ascontiguousarray(bias, dtype=np.float32)

    # host im2col: patches [B, N, C*K*K]
    win = np.lib.stride_tricks.sliding_window_view(x, (K, K), axis=(2, 3))
    r0 = centers[:, :, 0] - K // 2        # [B, N]
    c0 = centers[:, :, 1] - K // 2
    b_ids = np.arange(B)[:, None]
    patches = win[b_ids, :, r0, c0]       # [B, N, C, K, K]

    # weight [O, C, K, K] -> wT [KDIM, O] -> [128, KSL, O]
    wflat = weight.reshape(O, KDIM)
    wt_host = np.ascontiguousarray(
        wflat.T.reshape(KSL, P, O).transpose(1, 0, 2))
    bias_host = bias.reshape(1, O)

    in_maps = []
    for core in range(NCORES):
        pc = patches[core * B_LOC:(core + 1) * B_LOC].reshape(NPC, KDIM)
        # k-major: gt[p, ks, n] = patch n element ks*128+p
        gt_host = np.ascontiguousarray(
            pc.T.reshape(KSL, P, NPC).transpose(1, 0, 2))
        in_maps.append({"gt": gt_host, "wt": wt_host, "bias": bias_host})
    return in_maps


def kernel(x, centers, weight, bias):
    from concourse.bass_utils import run_bass_kernel_spmd
    nc = _get_nc(1)
    in_maps = _prep_inputs(x, centers, weight, bias)
    res = run_bass_kernel_spmd(nc, in_maps, list(range(NCORES))).results
    out = np.stack([res[i]["out"] for i in range(NCORES)], axis=0)
    return out.reshape(B, N, O)


# revision 8
# speedup vs baseline: 1.1768x; 1.0561x over previous
"""Trainium2 Bass kernel for nn_CustomConv2D: gather 16x16 patches at given
centers and apply a shared [768 -> 1024] linear projection + bias.

Sharding: data-parallel over batch across 8 NeuronCores (8 images/core,
4608 patches/core); weight+bias replicated.

Host prepares im2col patches in k-major layout ([128, 6, NPC]: contraction
on partitions); the device then runs a pure accumulating-matmul pipeline:
per 128-patch block, 12 matmuls [128x128 (f32r) @ 128x512] accumulate
out[128 patches, 1024] over the 6 k-slices in PSUM, bias-add on DVE, DMA
out. The patch extraction runs on host: TRN2's SWDGE indirect-DMA costs
~1.4us/instruction with one descriptor per partition (measured), so any
device-side gather of 221k 64B patch rows is ~2.4ms -- off the roofline,
while the projection itself is compute/HBM co-bound at ~100us/core.

Matmul dtype: float32r (TRN2 fast-fp32, 1 cycle/row vs 4 for fp32,
~1.5e-4 relative rounding). Set CONV_MM_DT=f32 for exact fp32 (4x PE).
"""

import os
import numpy as np

import concourse.bass as bass
from concourse import bacc
import concourse.mybir as mybir
import concourse.tile as tile

# problem shape (hardcoded per contract)
B, C, H, W = 64, 3, 384, 384
N, K, O = 576, 16, 1024
NCORES = 8
B_LOC = B // NCORES          # 8 images per core
NPC = B_LOC * N              # 4608 patches per core
P = 128                      # partitions / patches per block
NBLK = NPC // P              # 36 blocks
KDIM = C * K * K             # 768 contraction dim
KSL = KDIM // P              # 6 k-slices

MM_DT = os.environ.get("CONV_MM_DT", "f32r")


def _build(reps: int = 1):
    nc = bacc.Bacc()
    f32 = mybir.dt.float32
    mm_dt = f32 if MM_DT == "f32" else mybir.dt.float32r

    gt_t = nc.declare_dram_parameter("gt", [P, KSL, NPC], mm_dt, isOutput=False)
    wt_t = nc.declare_dram_parameter("wt", [P, KSL, O], mm_dt, isOutput=False)
    bias_t = nc.declare_dram_parameter("bias", [1, O], f32, isOutput=False)
    out_t = nc.declare_dram_parameter("out", [NPC, O], f32, isOutput=True)

    with tile.TileContext(nc) as tc:
        with (
            tc.tile_pool(name="const", bufs=1) as cpool,
            tc.tile_pool(name="gt", bufs=4) as gtpool,
            tc.tile_pool(name="osb", bufs=4) as opool,
            tc.tile_pool(name="outp", bufs=4, space="PSUM") as outppool,
        ):
            # weights chunked per (k-slice, half) on the SWDGE ring so the
            # first matmuls only wait for their own 256KB chunk and the
            # sync HWDGE ring stays dedicated to patch loads
            wt_sb = cpool.tile([P, KSL, O], mm_dt)
            for ks in range(KSL):
                for h in range(O // 512):
                    hs = slice(h * 512, (h + 1) * 512)
                    nc.gpsimd.dma_start(wt_sb[:, ks, hs], wt_t[:, ks, hs])
            bias_row = cpool.tile([1, O], f32)
            nc.gpsimd.dma_start(bias_row[:], bias_t[:])
            ones_col = cpool.tile([1, P], f32)
            nc.vector.memset(ones_col[:], 1.0)

            # bias broadcast to [128, O] via K=1 matmul
            bias_ps = outppool.tile([P, O], f32, tag="outp")
            for h in range(O // 512):
                nc.tensor.matmul(bias_ps[:, h * 512:(h + 1) * 512],
                                 lhsT=ones_col[:],
                                 rhs=bias_row[:, h * 512:(h + 1) * 512],
                                 start=True, stop=True)
            bias_bc = cpool.tile([P, O], f32)
            nc.vector.tensor_copy(bias_bc[:], bias_ps[:])

            def body(_i=None):
                for t in range(NBLK):
                    # k-major patch tile for this block: [128k, 6, 128p]
                    gt_sb = gtpool.tile([P, KSL, P], mm_dt, tag="gt")
                    nc.sync.dma_start(gt_sb[:], gt_t[:, :, t * P:(t + 1) * P])
                    out_ps = outppool.tile([P, O], f32, tag="outp")
                    for h in range(O // 512):
                        hs = slice(h * 512, (h + 1) * 512)
                        for ks in range(KSL):
                            nc.tensor.matmul(
                                out_ps[:, hs],
                                lhsT=gt_sb[:, ks, :],
                                rhs=wt_sb[:, ks, hs],
                                start=(ks == 0), stop=(ks == KSL - 1),
                            )
                    # bias-add + store per 512-half: the first half's store
                    # (on the second HWDGE ring) overlaps the second half's
                    # add, and stores stay off the patch-load ring
                    o_sb = opool.tile([P, O], f32, tag="osb")
                    for h in range(O // 512):
                        hs = slice(h * 512, (h + 1) * 512)
                        nc.vector.tensor_add(o_sb[:, hs], out_ps[:, hs],
                                             bias_bc[:, hs])
                        nc.scalar.dma_start(out_t[t * P:(t + 1) * P, hs],
                                            o_sb[:, hs])

            if reps == 1:
                body()
            else:
                with tc.For_i(0, reps, 1) as i:
                    body(i)
    nc.finalize()
    return nc


_CACHE = {}


def _get_nc(reps: int = 1):
    if reps not in _CACHE:
        _CACHE[reps] = _build(reps)
    return _CACHE[reps]


def _prep_inputs(x, centers, weight, bias):
    x = np.ascontiguousarray(x, dtype=np.float32)
    centers = np.asarray(centers, dtype=np.int64)
    weight = np.ascontiguousarray(weight, dtype=np.float32)
    bias = np.ascontiguousarray(bias, dtype=np.float32)

    # host im2col: patches [B, N, C*K*K]
    win = np.lib.stride_tricks.sliding_window_view(x, (K, K), axis=(2, 3))
    r0 = centers[:, :, 0] - K // 2        # [B, N]
    c0 = centers[:, :, 1] - K // 2
    b_ids = np.arange(B)[:, None]
    patches = win[b_ids, :, r0, c0]       # [B, N, C, K, K]

    # weight [O, C, K, K] -> wT [KDIM, O] -> [128, KSL, O]
    wflat = weight.reshape(O, KDIM)
    wt_host = np.ascontiguousarray(
        wflat.T.reshape(KSL, P, O).transpose(1, 0, 2))
    bias_host = bias.reshape(1, O)

    in_maps = []
    for core in range(NCORES):
        pc = patches[core * B_LOC:(core + 1) * B_LOC].reshape(NPC, KDIM)
        # k-major: gt[p, ks, n] = patch n element ks*128+p
        gt_host = np.ascontiguousarray(
            pc.T.reshape(KSL, P, NPC).transpose(1, 0, 2))
        in_maps.append({"gt": gt_host, "wt": wt_host, "bias": bias_host})
    return in_maps


def kernel(x, centers, weight, bias):
    from concourse.bass_utils import run_bass_kernel_spmd
    nc = _get_nc(1)
    in_maps = _prep_inputs(x, centers, weight, bias)
    res = run_bass_kernel_spmd(nc, in_maps, list(range(NCORES))).results
    out = np.stack([res[i]["out"] for i in range(NCORES)], axis=0)
    return out.reshape(B, N, O)


# revision 9
# speedup vs baseline: 1.2509x; 1.0629x over previous
"""Trainium2 Bass kernel for nn_CustomConv2D: gather 16x16 patches at given
centers and apply a shared [768 -> 1024] linear projection + bias.

Sharding: data-parallel over batch across 8 NeuronCores (8 images/core,
4608 patches/core); weight+bias replicated.

Host prepares im2col patches in k-major layout ([128, 6, NPC]: contraction
on partitions); the device then runs a pure accumulating-matmul pipeline:
per 128-patch block, 12 matmuls [128x128 (f32r) @ 128x512] accumulate
out[128 patches, 1024] over the 6 k-slices in PSUM, bias-add on DVE, DMA
out. The patch extraction runs on host: TRN2's SWDGE indirect-DMA costs
~1.4us/instruction with one descriptor per partition (measured), so any
device-side gather of 221k 64B patch rows is ~2.4ms -- off the roofline,
while the projection itself is compute/HBM co-bound at ~100us/core.

Matmul dtype: float32r (TRN2 fast-fp32, 1 cycle/row vs 4 for fp32,
~1.5e-4 relative rounding). Set CONV_MM_DT=f32 for exact fp32 (4x PE).
"""

import os
import numpy as np

import concourse.bass as bass
from concourse import bacc
import concourse.mybir as mybir
import concourse.tile as tile

# problem shape (hardcoded per contract)
B, C, H, W = 64, 3, 384, 384
N, K, O = 576, 16, 1024
NCORES = 8
B_LOC = B // NCORES          # 8 images per core
NPC = B_LOC * N              # 4608 patches per core
P = 128                      # partitions / patches per block
NBLK = NPC // P              # 36 blocks
KDIM = C * K * K             # 768 contraction dim
KSL = KDIM // P              # 6 k-slices

MM_DT = os.environ.get("CONV_MM_DT", "f32r")


def _build(reps: int = 1):
    nc = bacc.Bacc()
    f32 = mybir.dt.float32
    mm_dt = f32 if MM_DT == "f32" else mybir.dt.float32r

    gt_t = nc.declare_dram_parameter("gt", [P, KSL, NPC], mm_dt, isOutput=False)
    wt_t = nc.declare_dram_parameter("wt", [P, KSL, O], mm_dt, isOutput=False)
    bias_t = nc.declare_dram_parameter("bias", [1, O], f32, isOutput=False)
    out_t = nc.declare_dram_parameter("out", [NPC, O], f32, isOutput=True)

    with tile.TileContext(nc) as tc:
        with (
            tc.tile_pool(name="const", bufs=1) as cpool,
            tc.tile_pool(name="gt", bufs=4) as gtpool,
            tc.tile_pool(name="osb", bufs=4) as opool,
            tc.tile_pool(name="outp", bufs=4, space="PSUM") as outppool,
        ):
            # weights chunked per k-slice; slice 0 is issued before the first
            # patch load so the first matmuls start ~2us in, the rest follow
            # interleaved behind block 0's patches on the same ring
            wt_sb = cpool.tile([P, KSL, O], mm_dt)
            nc.sync.dma_start(wt_sb[:, 0, :], wt_t[:, 0, :])
            # bias broadcast [1,O] -> [128,O] on the (otherwise idle) GpSimd
            bias_row = cpool.tile([1, O], f32)
            nc.gpsimd.dma_start(bias_row[:], bias_t[:])
            bias_bc = cpool.tile([P, O], f32)
            nc.gpsimd.partition_broadcast(bias_bc[:], bias_row[:])

            def body(_i=None):
                for t in range(NBLK):
                    # k-major patch tile for this block: [128k, 6, 128p]
                    gt_sb = gtpool.tile([P, KSL, P], mm_dt, tag="gt")
                    nc.sync.dma_start(gt_sb[:], gt_t[:, :, t * P:(t + 1) * P])
                    if t == 0:
                        for ks in range(1, KSL):
                            nc.sync.dma_start(wt_sb[:, ks, :], wt_t[:, ks, :])
                    out_ps = outppool.tile([P, O], f32, tag="outp")
                    for h in range(O // 512):
                        hs = slice(h * 512, (h + 1) * 512)
                        for ks in range(KSL):
                            nc.tensor.matmul(
                                out_ps[:, hs],
                                lhsT=gt_sb[:, ks, :],
                                rhs=wt_sb[:, ks, hs],
                                start=(ks == 0), stop=(ks == KSL - 1),
                            )
                    # bias-add + store per 512-half: the first half's store
                    # (on the second HWDGE ring) overlaps the second half's
                    # add, and stores stay off the patch-load ring
                    o_sb = opool.tile([P, O], f32, tag="osb")
                    for h in range(O // 512):
                        hs = slice(h * 512, (h + 1) * 512)
                        nc.vector.tensor_add(o_sb[:, hs], out_ps[:, hs],
                                             bias_bc[:, hs])
                        nc.scalar.dma_start(out_t[t * P:(t + 1) * P, hs],
                                            o_sb[:, hs])

            if reps == 1:
                body()
            else:
                with tc.For_i(0, reps, 1) as i:
                    body(i)
    nc.finalize()
    return nc


_CACHE = {}


def _get_nc(reps: int = 1):
    if reps not in _CACHE:
        _CACHE[reps] = _build(reps)
    return _CACHE[reps]


def _prep_inputs(x, centers, weight, bias):
    x = np.ascontiguousarray(x, dtype=np.float32)
    centers = np.asarray(centers, dtype=np.int64)
    weight = np.ascontiguousarray(weight, dtype=np.float32)
    bias = np.ascontiguousarray(bias, dtype=np.float32)

    # host im2col: patches [B, N, C*K*K]
    win = np.lib.stride_tricks.sliding_window_view(x, (K, K), axis=(2, 3))
    r0 = centers[:, :, 0] - K // 2        # [B, N]
    c0 = centers[:, :, 1] - K // 2
    b_ids = np.arange(B)[:, None]
    patches = win[b_ids, :, r0, c0]       # [B, N, C, K, K]

    # weight [O, C, K, K] -> wT [KDIM, O] -> [128, KSL, O]
    wflat = weight.reshape(O, KDIM)
    wt_host = np.ascontiguousarray(
        wflat.T.reshape(KSL, P, O).transpose(1, 0, 2))
    bias_host = bias.reshape(1, O)

    in_maps = []
    for core in range(NCORES):
        pc = patches[core * B_LOC:(core + 1) * B_LOC].reshape(NPC, KDIM)
        # k-major: gt[p, ks, n] = patch n element ks*128+p
        gt_host = np.ascontiguousarray(
            pc.T.reshape(KSL, P, NPC).transpose(1, 0, 2))
        in_maps.append({"gt": gt_host, "wt": wt_host, "bias": bias_host})
    return in_maps


def kernel(x, centers, weight, bias):
    from concourse.bass_utils import run_bass_kernel_spmd
    nc = _get_nc(1)
    in_maps = _prep_inputs(x, centers, weight, bias)
    res = run_bass_kernel_spmd(nc, in_maps, list(range(NCORES))).results
    out = np.stack([res[i]["out"] for i in range(NCORES)], axis=0)
    return out.reshape(B, N, O)


# revision 10
# speedup vs baseline: 1.2883x; 1.0299x over previous
"""Trainium2 Bass kernel for nn_CustomConv2D: gather 16x16 patches at given
centers and apply a shared [768 -> 1024] linear projection + bias.

Sharding: data-parallel over batch across 8 NeuronCores (8 images/core,
4608 patches/core); weight+bias replicated.

Host prepares im2col patches in k-major layout ([128, 6, NPC]: contraction
on partitions); the device then runs a pure accumulating-matmul pipeline:
per 128-patch block, 12 matmuls [128x128 (f32r) @ 128x512] accumulate
out[128 patches, 1024] over the 6 k-slices in PSUM, bias-add on DVE, DMA
out. The patch extraction runs on host: TRN2's SWDGE indirect-DMA costs
~1.4us/instruction with one descriptor per partition (measured), so any
device-side gather of 221k 64B patch rows is ~2.4ms -- off the roofline,
while the projection itself is compute/HBM co-bound at ~100us/core.

Matmul dtype: float32r (TRN2 fast-fp32, 1 cycle/row vs 4 for fp32,
~1.5e-4 relative rounding). Set CONV_MM_DT=f32 for exact fp32 (4x PE).
"""

import os
import numpy as np

import concourse.bass as bass
from concourse import bacc
import concourse.mybir as mybir
import concourse.tile as tile

# problem shape (hardcoded per contract)
B, C, H, W = 64, 3, 384, 384
N, K, O = 576, 16, 1024
NCORES = 8
B_LOC = B // NCORES          # 8 images per core
NPC = B_LOC * N              # 4608 patches per core
P = 128                      # partitions / patches per block
NBLK = NPC // P              # 36 blocks
KDIM = C * K * K             # 768 contraction dim
KSL = KDIM // P              # 6 k-slices

MM_DT = os.environ.get("CONV_MM_DT", "f32r")


def _build(reps: int = 1):
    nc = bacc.Bacc()
    f32 = mybir.dt.float32
    mm_dt = f32 if MM_DT == "f32" else mybir.dt.float32r

    gt_t = nc.declare_dram_parameter("gt", [P, KSL, NPC], mm_dt, isOutput=False)
    wt_t = nc.declare_dram_parameter("wt", [P, KSL, O], mm_dt, isOutput=False)
    bias_t = nc.declare_dram_parameter("bias", [1, O], f32, isOutput=False)
    out_t = nc.declare_dram_parameter("out", [NPC, O], f32, isOutput=True)

    with tile.TileContext(nc) as tc:
        with (
            tc.tile_pool(name="const", bufs=1) as cpool,
            tc.tile_pool(name="gt", bufs=4) as gtpool,
            tc.tile_pool(name="osb", bufs=4) as opool,
            tc.tile_pool(name="outp", bufs=4, space="PSUM") as outppool,
        ):
            # weights chunked per k-slice; slice 0 is issued before the first
            # patch load so the first matmuls start ~2us in, the rest follow
            # interleaved behind block 0's patches on the same ring
            wt_sb = cpool.tile([P, KSL, O], mm_dt)
            nc.sync.dma_start(wt_sb[:, 0, :], wt_t[:, 0, :])
            # bias broadcast [1,O] -> [128,O] on the (otherwise idle) GpSimd
            bias_row = cpool.tile([1, O], f32)
            nc.gpsimd.dma_start(bias_row[:], bias_t[:])
            bias_bc = cpool.tile([P, O], f32)
            nc.gpsimd.partition_broadcast(bias_bc[:], bias_row[:])

            def body(_i=None):
                for t in range(NBLK):
                    # k-major patch tile for this block: [128k, 6, 128p]
                    gt_sb = gtpool.tile([P, KSL, P], mm_dt, tag="gt")
                    nc.sync.dma_start(gt_sb[:], gt_t[:, :, t * P:(t + 1) * P])
                    if t == 0:
                        for ks in range(1, KSL):
                            nc.sync.dma_start(wt_sb[:, ks, :], wt_t[:, ks, :])
                    out_ps = outppool.tile([P, O], f32, tag="outp")
                    # k-slice outer: both halves consume a weight chunk right
                    # after it lands, so block 0 isn't gated on the full tensor
                    for ks in range(KSL):
                        for h in range(O // 512):
                            hs = slice(h * 512, (h + 1) * 512)
                            nc.tensor.matmul(
                                out_ps[:, hs],
                                lhsT=gt_sb[:, ks, :],
                                rhs=wt_sb[:, ks, hs],
                                start=(ks == 0), stop=(ks == KSL - 1),
                            )
                    # bias-add + store per 512-half: the first half's store
                    # (on the second HWDGE ring) overlaps the second half's
                    # add, and stores stay off the patch-load ring
                    o_sb = opool.tile([P, O], f32, tag="osb")
                    for h in range(O // 512):
                        hs = slice(h * 512, (h + 1) * 512)
                        nc.vector.tensor_add(o_sb[:, hs], out_ps[:, hs],
                                             bias_bc[:, hs])
                        nc.scalar.dma_start(out_t[t * P:(t + 1) * P, hs],
                                            o_sb[:, hs])

            if reps == 1:
                body()
            else:
                with tc.For_i(0, reps, 1) as i:
                    body(i)
    nc.finalize()
    return nc


_CACHE = {}


def _get_nc(reps: int = 1):
    if reps not in _CACHE:
        _CACHE[reps] = _build(reps)
    return _CACHE[reps]


def _prep_inputs(x, centers, weight, bias):
    x = np.ascontiguousarray(x, dtype=np.float32)
    centers = np.asarray(centers, dtype=np.int64)
    weight = np.ascontiguousarray(weight, dtype=np.float32)
    bias = np.ascontiguousarray(bias, dtype=np.float32)

    # host im2col: patches [B, N, C*K*K]
    win = np.lib.stride_tricks.sliding_window_view(x, (K, K), axis=(2, 3))
    r0 = centers[:, :, 0] - K // 2        # [B, N]
    c0 = centers[:, :, 1] - K // 2
    b_ids = np.arange(B)[:, None]
    patches = win[b_ids, :, r0, c0]       # [B, N, C, K, K]

    # weight [O, C, K, K] -> wT [KDIM, O] -> [128, KSL, O]
    wflat = weight.reshape(O, KDIM)
    wt_host = np.ascontiguousarray(
        wflat.T.reshape(KSL, P, O).transpose(1, 0, 2))
    bias_host = bias.reshape(1, O)

    in_maps = []
    for core in range(NCORES):
        pc = patches[core * B_LOC:(core + 1) * B_LOC].reshape(NPC, KDIM)
        # k-major: gt[p, ks, n] = patch n element ks*128+p
        gt_host = np.ascontiguousarray(
            pc.T.reshape(KSL, P, NPC).transpose(1, 0, 2))
        in_maps.append({"gt": gt_host, "wt": wt_host, "bias": bias_host})
    return in_maps


def kernel(x, centers, weight, bias):
    from concourse.bass_utils import run_bass_kernel_spmd
    nc = _get_nc(1)
    in_maps = _prep_inputs(x, centers, weight, bias)
    res = run_bass_kernel_spmd(nc, in_maps, list(range(NCORES))).results
    out = np.stack([res[i]["out"] for i in range(NCORES)], axis=0)
    return out.reshape(B, N, O)
